# revision 65
# baseline (speedup 1.0000x reference)
"""Multi-head causal attention (B=1024, T=64, C=768, H=12, D=64) on 8 TRN2
NeuronCores, data-parallel over the batch dimension (128 batches/core).

Dataflow per core (fp8/bf16 matmuls, fp32 PSUM accumulate):
  - Host pre-stages layouts: x is transposed per core to xT [c, tok] in
    bf16 (for V) and fp8-e4m3 (for Q/K); weights arrive transposed as
    wT[p, cc, m] = W[m, cc*128+p] - Wq/Wk in fp8 scaled x16 (the 256x
    score factor is divided out in the exp scale), Wv pre-split by head
    parity, Wp/bias in bf16/f32.
  - QT/KT [hd, tok] = WT.T @ XT as fp8 DoubleRow matmuls (256-deep
    contraction per pass, ~1.4x bf16 throughput, N=512 moving). K is
    scattered into a per-(head-pair, batch) block-diagonal ktbd.
  - V is computed directly in block layout with a column-tiled matmul
    pair (even heads -> psum rows 0:64, odd heads -> rows 64:128 via
    tile_position=(0,64)), then copied next to per-head ones columns in
    vbd so the softmax denominator falls out of the AV matmul for free.
  - Per (batch, head-pair): scoresT = ktbd.T @ QT; Pexp = exp(s/2048) *
    causal mask; Y[t, (h,d)|den] = Pexp.T @ vbd; normalize by the
    reciprocal of the den column; PE-transpose Y -> YT (bf16 psum);
    out = YT.T @ WpT + bp.
  - Chunks are double-buffered (parity tiles) so chunk N+1 load/proj
    overlaps chunk N attention and the PE never idles long enough to
    re-throttle (HAM). PSUM is split into five dedicated pools
    (scores/Y/out/Y^T/proj) so pipeline stages never alias banks.
"""

import os
import numpy as np

os.environ.setdefault("NEURON_RT_RESET_CORES", "1")

P = 128
B, T, C, H, Dh = 1024, 64, 768, 12, 64
HD = H * Dh            # 768
NCC = C // P           # 6 contraction chunks
NHD = HD // P          # 6 hd chunks (head pairs)
N_CORES = 8

_cache = {}


def _patch_tile_drain(tile, mybir):
    """walrus CTRL (Drain) ops in this toolchain accept only 1 sem-wait;
    spread the TileContext exit-drain's waits across preceding SP nops."""
    from concourse.vector_clock import ScopedClock

    if getattr(tile.TileContext, "_drain_patched", False):
        return

    def _drain_and_barrier(self, tick_clock, wait_clock):
        nc = self.nc
        drain_inst = nc.sync.drain()
        wait_clock.add_sem_waits(
            drain_inst.ins, ScopedClock({None: tick_clock.global_clock})
        )
        waits = list(drain_inst.ins.sync_info.on_wait)
        if len(waits) > 1:
            drain_inst.ins.sync_info.on_wait = waits[-1:]
            cur_bb = nc.cur_bb.bb
            idx = cur_bb.instructions.index(drain_inst.ins)
            extra = []
            for w in waits[:-1]:
                nop = mybir.InstNoOp(name=f"I-{nc.next_id()}", ins=[], outs=[])
                nop.engine = drain_inst.ins.engine
                nop.sync_info = mybir.SyncInfo(on_wait=[w], on_update=[])
                nc.register_instruction(nop)
                extra.append(nop)
            cur_bb.instructions[idx:idx] = extra
        nc.all_engine_barrier()
        assert self.sems is not None
        popped = nc._tile_sem_poison_stack.pop()
        assert popped is self._sem_poison
        nc.clear_and_free_semaphores(list(self.sems.allocated().values()))
        nc.all_engine_barrier()

    tile.TileContext._drain_and_barrier = _drain_and_barrier
    tile.TileContext._drain_patched = True


def _install_loud_cc_hook():
    """Surface real exceptions from the neuronx_cc hook (C wrapper eats them)."""
    from concourse import bass2jax as _b2j
    if getattr(_b2j, "_loud_hook_installed", False):
        return
    _orig = _b2j.neuronx_cc_hook
    def _loud(*a, **k):
        try:
            return _orig(*a, **k)
        except BaseException:
            import traceback
            traceback.print_exc()
            raise
    _b2j.neuronx_cc_hook = _loud
    _b2j._loud_hook_installed = True


def _split_multi_waits(nc, mybir, K=1):
    """This walrus build supports only one sem-wait per instruction: move
    excess waits onto same-engine NOPs inserted directly before the owner."""
    def fix_block(bb):
        insts = bb.instructions
        i = 0
        while i < len(insts):
            ins = insts[i]
            si = ins.sync_info
            w = list(si.on_wait) if si is not None and si.on_wait else []
            if len(w) > K:
                carriers = []
                for j in range(0, len(w) - K, K):
                    nop = mybir.InstNoOp(name=f"I-{nc.next_id()}", ins=[], outs=[])
                    nop.engine = ins.engine
                    nop.sync_info = mybir.SyncInfo(on_wait=w[j:j + K], on_update=[])
                    nc.register_instruction(nop)
                    carriers.append(nop)
                si.on_wait = w[len(w) - K:]
                insts[i:i] = carriers
                i += len(carriers)
            i += 1
    for fn in nc.m.functions:
        for bb in fn.blocks:
            fix_block(bb)


def _bp_bcast_ap(bass, bp_d):
    a = bp_d[:]
    return bass.AP(tensor=a.tensor, offset=a.offset, ap=[[0, P]] + list(a.ap))


def build_nc(B_loc=B // N_CORES, chunk_tok=512, debug=False):
    import concourse.bass as bass
    import concourse.tile as tile
    from concourse import mybir
    from contextlib import ExitStack

    _patch_tile_drain(tile, mybir)
    _install_loud_cc_hook()

    F32 = mybir.dt.float32
    BF16 = mybir.dt.bfloat16
    AF = mybir.ActivationFunctionType
    ALU = mybir.AluOpType

    BT = B_loc * T
    chunk_tok = min(chunk_tok, BT)
    n_chunks = BT // chunk_tok
    assert n_chunks * chunk_tok == BT
    TT = chunk_tok // P     # 128-token tiles per chunk
    NB = chunk_tok // T     # batches per chunk

    F8 = mybir.dt.float8e4
    nc = bass.Bass()
    x_d = nc.declare_dram_parameter("xT", [C, BT], BF16, isOutput=False)
    x8_d = nc.declare_dram_parameter("xT8", [C, BT], F8, isOutput=False)
    wq_d = nc.declare_dram_parameter("wqT", [P, NCC, HD], F8, isOutput=False)
    wk_d = nc.declare_dram_parameter("wkT", [P, NCC, HD], F8, isOutput=False)
    wv_d = nc.declare_dram_parameter("wvT", [P, NCC, 2, NHD, Dh], BF16, isOutput=False)
    wp_d = nc.declare_dram_parameter("wpT", [P, NHD, C], BF16, isOutput=False)
    bp_d = nc.declare_dram_parameter("bp", [C], F32, isOutput=False)
    id_d = nc.declare_dram_parameter("ident", [P, P], BF16, isOutput=False)
    mk_d = nc.declare_dram_parameter("mask", [P, T], BF16, isOutput=False)
    kz_d = nc.declare_dram_parameter("ktbd_init", [P, NHD, NB, P], BF16,
                                     isOutput=False)
    vz_d = nc.declare_dram_parameter("vbd_init", [P, NHD, NB, 2 * (Dh + 1)],
                                     BF16, isOutput=False)
    out_d = nc.declare_dram_parameter("out", [B_loc, T, C], F32, isOutput=True)

    of = out_d[:].flatten_outer_dims()    # [BT, C]

    with tile.TileContext(nc) as tc, ExitStack() as ctx:
        sing = ctx.enter_context(tc.tile_pool(name="sing", bufs=1))
        ostage = ctx.enter_context(tc.tile_pool(name="ostage", bufs=4))
        pexp_p = ctx.enter_context(tc.tile_pool(name="pexp", bufs=4))
        yb_p = ctx.enter_context(tc.tile_pool(name="yb", bufs=4))
        yt_p = ctx.enter_context(tc.tile_pool(name="yt", bufs=4))
        small = ctx.enter_context(tc.tile_pool(name="small", bufs=6))
        # PSUM: 8 banks total: s(1) + y(2) + o(2) + yt(1, bf16) + proj(2)
        ps_s = ctx.enter_context(tc.tile_pool(name="ps_s", bufs=1, space="PSUM"))
        ps_y = ctx.enter_context(tc.tile_pool(name="ps_y", bufs=2, space="PSUM"))
        ps_o = ctx.enter_context(tc.tile_pool(name="ps_o", bufs=2, space="PSUM"))
        ps_t = ctx.enter_context(tc.tile_pool(name="ps_t", bufs=1, space="PSUM"))
        ps_p = ctx.enter_context(tc.tile_pool(name="ps_p", bufs=2, space="PSUM"))

        def ptile(pool, pdim, shape, name, dt=None):
            # slot is always one full PSUM bank (2KB per partition)
            width = 512 if dt is None else 1024
            t = pool.tile([P, width], dt or F32, tag="ps", name=name)
            flat = t[:pdim, : int(np.prod(shape[1:]))]
            return flat.rearrange(
                "p (a b) -> p a b", a=shape[1]
            ) if len(shape) == 3 else flat

        # ---- constants; Q/K weights first (they gate chunk-0 matmuls) ----
        wqT = sing.tile([P, NCC, HD], F8, name="wqT")
        nc.sync.dma_start(out=wqT, in_=wq_d[:])
        wkT = sing.tile([P, NCC, HD], F8, name="wkT")
        nc.sync.dma_start(out=wkT, in_=wk_d[:])
        id_sb = sing.tile([P, P], BF16)
        nc.gpsimd.dma_start(out=id_sb, in_=id_d[:])
        mask_sb = sing.tile([P, T], BF16)
        nc.gpsimd.dma_start(out=mask_sb, in_=mk_d[:])
        bp_bc = sing.tile([P, C], F32)
        nc.gpsimd.dma_start(out=bp_bc, in_=_bp_bcast_ap(bass, bp_d))
        wvT = sing.tile([P, NCC, 2, NHD, Dh], BF16, name="wvT")
        nc.gpsimd.dma_start(out=wvT, in_=wv_d[:])
        wpT = sing.tile([P, NHD, C], BF16, name="wpT")
        nc.gpsimd.dma_start(out=wpT, in_=wp_d[:])

        # ---- double-buffered per-chunk tensors ----
        def mk2(shape, name):
            return [sing.tile(shape, BF16, name=f"{name}{i}") for i in range(2)]

        xT2 = mk2([P, NCC, chunk_tok], "xT")
        x8T2 = [sing.tile([P, NCC, chunk_tok], F8, name=f"x8T{i}")
                for i in range(2)]
        qT2 = mk2([P, NHD, chunk_tok], "qT")
        ktbd2 = mk2([P, NHD, NB, P], "ktbd")
        vbd2 = mk2([P, NHD, NB, 2 * (Dh + 1)], "vbd")
        # structural zeros/ones come pre-baked from the host; buffer 0 is
        # chunk-0-critical, buffer 1 only matters by chunk 1
        nc.scalar.dma_start(out=ktbd2[0], in_=kz_d[:])
        nc.scalar.dma_start(out=vbd2[0], in_=vz_d[:])
        nc.gpsimd.dma_start(out=ktbd2[1], in_=kz_d[:])
        nc.gpsimd.dma_start(out=vbd2[1], in_=vz_d[:])

        for ci in range(n_chunks):
            tok0 = ci * chunk_tok
            xT = xT2[ci % 2]
            x8T = x8T2[ci % 2]
            qT = qT2[ci % 2]
            ktbd = ktbd2[ci % 2]
            vbd = vbd2[ci % 2]

            # ---- P0: load pre-transposed X chunk (fp8 for QK, bf16 for V) ----
            for cc in range(NCC):
                nc.sync.dma_start(
                    out=x8T[:, cc, :],
                    in_=x8_d[cc * P:(cc + 1) * P, tok0:tok0 + chunk_tok])
            for cc in range(NCC):
                nc.sync.dma_start(
                    out=xT[:, cc, :],
                    in_=x_d[cc * P:(cc + 1) * P, tok0:tok0 + chunk_tok])

            # ---- P1a: KT / QT projections (fp8 DoubleRow, N=chunk) ----
            for wT, dst in ((wkT, "k"), (wqT, "q")):
                for m in range(NHD):
                    pss = ptile(ps_p, P, (P, chunk_tok), f"proj_{dst}{m}")
                    for cb in range(NCC // 2):
                        nc.tensor.matmul(
                            pss, wT[:, 2 * cb:2 * cb + 2, m * P:(m + 1) * P],
                            x8T[:, 2 * cb:2 * cb + 2, :],
                            start=(cb == 0), stop=(cb == NCC // 2 - 1),
                            perf_mode=mybir.MatmulPerfMode.DoubleRow)
                    if dst == "q":
                        nc.scalar.copy(out=qT[:, m, :], in_=pss)
                    else:
                        # split the two block-diag copies across engines
                        nc.scalar.copy(
                            out=ktbd[0:T, m, :, 0:T],
                            in_=pss[0:T].rearrange("p (nb t) -> p nb t", nb=NB))
                        nc.vector.tensor_copy(
                            out=ktbd[T:P, m, :, T:P],
                            in_=pss[T:P].rearrange("p (nb t) -> p nb t", nb=NB))

            # ---- P1b: V directly in block layout (col-tiled matmul pair) ----
            vbd_v = vbd.rearrange("p a nb (two c) -> p a nb two c", two=2)
            for b in range(NB):
                vpsE = ptile(ps_p, P, (P, NHD, Dh), f"vpsE{b}")
                vpsO = ptile(ps_p, P, (P, NHD, Dh), f"vpsO{b}")
                lhs = xT[:, :, b * T:(b + 1) * T]
                for cc in range(NCC):
                    nc.tensor.matmul(
                        vpsE[0:T], lhs[:, cc, :], wvT[:, cc, 0],
                        start=(cc == 0), stop=(cc == NCC - 1))
                    nc.tensor.matmul(
                        vpsO[T:P], lhs[:, cc, :], wvT[:, cc, 1],
                        start=(cc == 0), stop=(cc == NCC - 1),
                        tile_position=(0, T))
                nc.vector.tensor_copy(
                    out=vbd_v[0:T, :, b, 0, 0:Dh], in_=vpsE[0:T])
                nc.vector.tensor_copy(
                    out=vbd_v[T:P, :, b, 1, 0:Dh], in_=vpsO[T:P])

            # ---- P2+P3: attention, Y PE-transpose, output projection ----
            for it in range(TT):
                yb = yb_p.tile([P, HD], BF16, tag="yb")
                pex = pexp_p.tile([P, 2, NHD, T], BF16, tag="pex", name="pex")
                y_ps = [ptile(ps_y, P, (P, 3, 2 * (Dh + 1)), f"y_ps{h2}")
                        for h2 in range(2)]
                for half in range(2):          # two batches per 128-token tile
                    b = it * 2 + half
                    prow = half * T
                    s_ps = ptile(ps_s, P, (P, NHD, T), f"s_ps{half}")
                    for p_ in range(NHD):
                        nc.tensor.matmul(
                            s_ps[:, p_, :],
                            ktbd[:, p_, b, :],
                            qT[:, p_, b * T:(b + 1) * T],
                            start=True, stop=True)
                    # q,k carry a 16x host-side weight scale each: 0.125/256.
                    # Exp/mask are split in two head-pair slices so the AV
                    # matmuls of pairs 0-2 issue while pairs 3-5 still exp.
                    for hh in range(2):
                        sl = slice(hh * 3, (hh + 1) * 3)
                        nc.scalar.activation(
                            out=pex[:, half, sl], in_=s_ps[:, sl, :],
                            func=AF.Exp, scale=0.125 / 256.0)
                        nc.vector.tensor_tensor(
                            pex[:, half, sl], pex[:, half, sl],
                            mask_sb[:, None, :].to_broadcast([P, 3, T]),
                            ALU.mult)
                    for p_ in range(NHD):
                        nc.tensor.matmul(
                            y_ps[p_ // 3][prow:prow + T, p_ % 3, :],
                            pex[:, half, p_, :],
                            vbd[:, p_, b, :],
                            start=True, stop=True)
                for h2 in range(2):
                    y_v = y_ps[h2].rearrange("p a (two c) -> p a two c", c=Dh + 1)
                    rec = small.tile([P, 3, 2, 1], F32, tag="rec", name="rec")
                    nc.vector.reciprocal(out=rec, in_=y_v[:, :, :, Dh:Dh + 1])
                    nc.vector.tensor_tensor(
                        yb[:, h2 * 384:(h2 + 1) * 384]
                            .rearrange("p (a two b) -> p a two b", a=3, two=2),
                        y_v[:, :, :, 0:Dh],
                        rec.to_broadcast([P, 3, 2, Dh]),
                        ALU.mult)
                # Y transpose on PE into one bf16 psum bank
                yt_ps = ptile(ps_t, P, (P, NHD, P), "yt_ps", BF16)
                for j in range(NHD):
                    nc.tensor.transpose(
                        yt_ps[:, j, :], yb[:, j * P:(j + 1) * P], id_sb)
                ytile = yt_p.tile([P, NHD, P], BF16, tag="ytile")
                nc.scalar.copy(out=ytile, in_=yt_ps)
                # output projection
                oA = ptile(ps_o, P, (P, 512), "o_psA")
                oB = ptile(ps_o, P, (P, 256), "o_psB")
                for j in range(NHD):
                    lhs = ytile[:, j, :]
                    nc.tensor.matmul(oA, lhs, wpT[:, j, 0:512],
                                     start=(j == 0), stop=(j == NHD - 1))
                    nc.tensor.matmul(oB, lhs, wpT[:, j, 512:768],
                                     start=(j == 0), stop=(j == NHD - 1))
                osb = ostage.tile([P, C], F32, tag="osb")
                nc.vector.tensor_tensor(osb[:, 0:512], oA, bp_bc[:, 0:512], ALU.add)
                nc.vector.tensor_tensor(osb[:, 512:768], oB, bp_bc[:, 512:768], ALU.add)
                row0 = tok0 + it * P
                nc.sync.dma_start(out=of[row0:row0 + P, :], in_=osb)

    _split_multi_waits(nc, mybir)
    return nc


def _get_program(B_loc, chunk_tok):
    key = (B_loc, chunk_tok)
    if key not in _cache:
        _cache[key] = build_nc(B_loc, chunk_tok)
    return _cache[key]


def make_const_inputs():
    import ml_dtypes
    ident = np.eye(P, dtype=ml_dtypes.bfloat16)
    # mask[s, t] = 1 if s <= t (causal, scoresT layout)
    m = np.tril(np.ones((T, T), dtype=np.float32)).T.astype(ml_dtypes.bfloat16)
    mask = np.vstack([m, m])   # replicated for both head partition-halves
    return ident, mask


def _prep_wT(W):
    """W [768(out), 768(in)] -> wT[p, cc, m] = W[m, cc*128+p], bf16."""
    import ml_dtypes
    w = np.ascontiguousarray(W, dtype=np.float32).reshape(HD, NCC, P)
    return np.ascontiguousarray(
        w.transpose(2, 1, 0)).astype(ml_dtypes.bfloat16)


def prepare(x, Wq, Wk, Wv, Wp, bp, chunk_tok=512):
    import ml_dtypes
    F8NP = ml_dtypes.float8_e4m3
    B_loc = B // N_CORES
    ident, mask = make_const_inputs()
    # Q/K weights in fp8 e4m3, scaled x16 to land in e4m3's normal range;
    # the 16*16 factor is divided back out in the exp() scale.
    w = np.ascontiguousarray(Wq.reshape(HD, C), dtype=np.float32).reshape(HD, NCC, P)
    wqT = np.ascontiguousarray((w * 16.0).transpose(2, 1, 0)).astype(F8NP)
    w = np.ascontiguousarray(Wk.reshape(HD, C), dtype=np.float32).reshape(HD, NCC, P)
    wkT = np.ascontiguousarray((w * 16.0).transpose(2, 1, 0)).astype(F8NP)
    # wvT pre-split by head parity: [p, cc, par, hp, d]
    wvT = _prep_wT(Wv.reshape(HD, C)).reshape(P, NCC, NHD, 2, Dh)
    wvT = np.ascontiguousarray(wvT.transpose(0, 1, 3, 2, 4))
    wpT = _prep_wT(Wp)   # Wp [C_out, HD_in]: contraction on hd
    # per-core x: [B_loc*T, C] -> transposed [C, B_loc*T], bf16 + fp8 copies
    xr = np.asarray(x, dtype=np.float32).reshape(N_CORES, B_loc * T, C)
    xTf = np.ascontiguousarray(xr.transpose(0, 2, 1))
    xTh = xTf.astype(ml_dtypes.bfloat16)
    xT8 = xTf.astype(F8NP)
    # host-baked structural init for the block-diagonal K / V-aug tiles
    NB = chunk_tok // T
    ktbd_init = np.zeros((P, NHD, NB, P), dtype=ml_dtypes.bfloat16)
    vbd_init = np.zeros((P, NHD, NB, 2 * (Dh + 1)), dtype=ml_dtypes.bfloat16)
    vbd_init[0:T, :, :, Dh] = 1.0
    vbd_init[T:P, :, :, 2 * Dh + 1] = 1.0
    nc = _get_program(B_loc, chunk_tok)
    in_maps = []
    for c in range(N_CORES):
        in_maps.append({
            "xT": xTh[c], "xT8": xT8[c],
            "wqT": wqT, "wkT": wkT, "wvT": wvT, "wpT": wpT,
            "bp": np.ascontiguousarray(bp, dtype=np.float32),
            "ident": ident,
            "mask": mask,
            "ktbd_init": ktbd_init, "vbd_init": vbd_init,
        })
    return nc, in_maps


def kernel(x, Wq, Wk, Wv, Wp, bp):
    from concourse import bass_utils

    nc, in_maps = prepare(x, Wq, Wk, Wv, Wp, bp)
    res = bass_utils.run_bass_kernel_spmd(nc, in_maps, list(range(N_CORES)))
    return np.concatenate([res.results[c]["out"] for c in range(N_CORES)], axis=0)


# revision 66
# speedup vs baseline: 1.0821x; 1.0821x over previous
"""Multi-head causal attention (B=1024, T=64, C=768, H=12, D=64) on 8 TRN2
NeuronCores, data-parallel over the batch dimension (128 batches/core).

Dataflow per core (fp8/bf16 matmuls, fp32 PSUM accumulate):
  - Host pre-stages layouts: x is transposed per core to xT [c, tok] in
    bf16 (for V) and fp8-e4m3 (for Q/K); weights arrive transposed as
    wT[p, cc, m] = W[m, cc*128+p] - Wq/Wk in fp8 scaled x16 (the 256x
    score factor is divided out in the exp scale), Wv pre-split by head
    parity, Wp/bias in bf16/f32.
  - QT/KT [hd, tok] = WT.T @ XT as fp8 DoubleRow matmuls (256-deep
    contraction per pass, ~1.4x bf16 throughput, N=512 moving). K is
    scattered into a per-(head-pair, batch) block-diagonal ktbd.
  - V is computed directly in block layout with a column-tiled matmul
    pair (even heads -> psum rows 0:64, odd heads -> rows 64:128 via
    tile_position=(0,64)), then copied next to per-head ones columns in
    vbd so the softmax denominator falls out of the AV matmul for free.
  - Per (batch, head-pair): scoresT = ktbd.T @ QT; Pexp = exp(s/2048) *
    causal mask; Y[t, (h,d)|den] = Pexp.T @ vbd; normalize by the
    reciprocal of the den column; PE-transpose Y -> YT (bf16 psum);
    out = YT.T @ WpT + bp.
  - Chunks are double-buffered (parity tiles) so chunk N+1 load/proj
    overlaps chunk N attention and the PE never idles long enough to
    re-throttle (HAM). PSUM is split into five dedicated pools
    (scores/Y/out/Y^T/proj) so pipeline stages never alias banks.
"""

import os
import numpy as np

os.environ.setdefault("NEURON_RT_RESET_CORES", "1")

P = 128
B, T, C, H, Dh = 1024, 64, 768, 12, 64
HD = H * Dh            # 768
NCC = C // P           # 6 contraction chunks
NHD = HD // P          # 6 hd chunks (head pairs)
N_CORES = 8

_cache = {}


def _patch_tile_drain(tile, mybir):
    """walrus CTRL (Drain) ops in this toolchain accept only 1 sem-wait;
    spread the TileContext exit-drain's waits across preceding SP nops."""
    from concourse.vector_clock import ScopedClock

    if getattr(tile.TileContext, "_drain_patched", False):
        return

    def _drain_and_barrier(self, tick_clock, wait_clock):
        nc = self.nc
        drain_inst = nc.sync.drain()
        wait_clock.add_sem_waits(
            drain_inst.ins, ScopedClock({None: tick_clock.global_clock})
        )
        waits = list(drain_inst.ins.sync_info.on_wait)
        if len(waits) > 1:
            drain_inst.ins.sync_info.on_wait = waits[-1:]
            cur_bb = nc.cur_bb.bb
            idx = cur_bb.instructions.index(drain_inst.ins)
            extra = []
            for w in waits[:-1]:
                nop = mybir.InstNoOp(name=f"I-{nc.next_id()}", ins=[], outs=[])
                nop.engine = drain_inst.ins.engine
                nop.sync_info = mybir.SyncInfo(on_wait=[w], on_update=[])
                nc.register_instruction(nop)
                extra.append(nop)
            cur_bb.instructions[idx:idx] = extra
        nc.all_engine_barrier()
        assert self.sems is not None
        popped = nc._tile_sem_poison_stack.pop()
        assert popped is self._sem_poison
        nc.clear_and_free_semaphores(list(self.sems.allocated().values()))
        nc.all_engine_barrier()

    tile.TileContext._drain_and_barrier = _drain_and_barrier
    tile.TileContext._drain_patched = True


def _install_loud_cc_hook():
    """Surface real exceptions from the neuronx_cc hook (C wrapper eats them)."""
    from concourse import bass2jax as _b2j
    if getattr(_b2j, "_loud_hook_installed", False):
        return
    _orig = _b2j.neuronx_cc_hook
    def _loud(*a, **k):
        try:
            return _orig(*a, **k)
        except BaseException:
            import traceback
            traceback.print_exc()
            raise
    _b2j.neuronx_cc_hook = _loud
    _b2j._loud_hook_installed = True


def _split_multi_waits(nc, mybir, K=1):
    """This walrus build supports only one sem-wait per instruction: move
    excess waits onto same-engine NOPs inserted directly before the owner."""
    def fix_block(bb):
        insts = bb.instructions
        i = 0
        while i < len(insts):
            ins = insts[i]
            si = ins.sync_info
            w = list(si.on_wait) if si is not None and si.on_wait else []
            if len(w) > K:
                carriers = []
                for j in range(0, len(w) - K, K):
                    nop = mybir.InstNoOp(name=f"I-{nc.next_id()}", ins=[], outs=[])
                    nop.engine = ins.engine
                    nop.sync_info = mybir.SyncInfo(on_wait=w[j:j + K], on_update=[])
                    nc.register_instruction(nop)
                    carriers.append(nop)
                si.on_wait = w[len(w) - K:]
                insts[i:i] = carriers
                i += len(carriers)
            i += 1
    for fn in nc.m.functions:
        for bb in fn.blocks:
            fix_block(bb)


def _bp_bcast_ap(bass, bp_d):
    a = bp_d[:]
    return bass.AP(tensor=a.tensor, offset=a.offset, ap=[[0, P]] + list(a.ap))


def build_nc(B_loc=B // N_CORES, chunk_tok=512, debug=False):
    import concourse.bass as bass
    import concourse.tile as tile
    from concourse import mybir
    from contextlib import ExitStack

    _patch_tile_drain(tile, mybir)
    _install_loud_cc_hook()

    F32 = mybir.dt.float32
    BF16 = mybir.dt.bfloat16
    AF = mybir.ActivationFunctionType
    ALU = mybir.AluOpType

    BT = B_loc * T
    chunk_tok = min(chunk_tok, BT)
    n_chunks = BT // chunk_tok
    assert n_chunks * chunk_tok == BT
    TT = chunk_tok // P     # 128-token tiles per chunk
    NB = chunk_tok // T     # batches per chunk

    F8 = mybir.dt.float8e4
    nc = bass.Bass()
    x_d = nc.declare_dram_parameter("xT", [C, BT], BF16, isOutput=False)
    x8_d = nc.declare_dram_parameter("xT8", [C, BT], F8, isOutput=False)
    wq_d = nc.declare_dram_parameter("wqT", [P, NCC, HD], F8, isOutput=False)
    wk_d = nc.declare_dram_parameter("wkT", [P, NCC, HD], F8, isOutput=False)
    wv_d = nc.declare_dram_parameter("wvT", [P, NCC, 2, NHD, Dh], BF16, isOutput=False)
    wp_d = nc.declare_dram_parameter("wpT", [P, NHD, C], BF16, isOutput=False)
    bp_d = nc.declare_dram_parameter("bp", [C], F32, isOutput=False)
    id_d = nc.declare_dram_parameter("ident", [P, P], BF16, isOutput=False)
    mk_d = nc.declare_dram_parameter("mask", [P, T], BF16, isOutput=False)
    kz_d = nc.declare_dram_parameter("ktbd_init", [P, NHD, NB, P], BF16,
                                     isOutput=False)
    vz_d = nc.declare_dram_parameter("vbd_init", [P, NHD, NB, 2 * (Dh + 1)],
                                     BF16, isOutput=False)
    out_d = nc.declare_dram_parameter("out", [B_loc, T, C], F32, isOutput=True)

    of = out_d[:].flatten_outer_dims()    # [BT, C]

    with tile.TileContext(nc) as tc, ExitStack() as ctx:
        sing = ctx.enter_context(tc.tile_pool(name="sing", bufs=1))
        ostage = ctx.enter_context(tc.tile_pool(name="ostage", bufs=4))
        pexp_p = ctx.enter_context(tc.tile_pool(name="pexp", bufs=4))
        yb_p = ctx.enter_context(tc.tile_pool(name="yb", bufs=4))
        yt_p = ctx.enter_context(tc.tile_pool(name="yt", bufs=4))
        small = ctx.enter_context(tc.tile_pool(name="small", bufs=6))
        # PSUM: 8 banks total: s(1) + y(2) + o(2) + yt(1, bf16) + proj(2)
        ps_s = ctx.enter_context(tc.tile_pool(name="ps_s", bufs=1, space="PSUM"))
        ps_y = ctx.enter_context(tc.tile_pool(name="ps_y", bufs=2, space="PSUM"))
        ps_o = ctx.enter_context(tc.tile_pool(name="ps_o", bufs=2, space="PSUM"))
        ps_t = ctx.enter_context(tc.tile_pool(name="ps_t", bufs=1, space="PSUM"))
        ps_p = ctx.enter_context(tc.tile_pool(name="ps_p", bufs=2, space="PSUM"))

        def ptile(pool, pdim, shape, name, dt=None):
            # slot is always one full PSUM bank (2KB per partition)
            width = 512 if dt is None else 1024
            t = pool.tile([P, width], dt or F32, tag="ps", name=name)
            flat = t[:pdim, : int(np.prod(shape[1:]))]
            return flat.rearrange(
                "p (a b) -> p a b", a=shape[1]
            ) if len(shape) == 3 else flat

        # ---- constants; Q/K weights first (they gate chunk-0 matmuls) ----
        wqT = sing.tile([P, NCC, HD], F8, name="wqT")
        nc.sync.dma_start(out=wqT, in_=wq_d[:])
        wkT = sing.tile([P, NCC, HD], F8, name="wkT")
        nc.sync.dma_start(out=wkT, in_=wk_d[:])
        id_sb = sing.tile([P, P], BF16)
        nc.gpsimd.dma_start(out=id_sb, in_=id_d[:])
        mask_sb = sing.tile([P, T], BF16)
        nc.gpsimd.dma_start(out=mask_sb, in_=mk_d[:])
        bp_bc = sing.tile([P, C], F32)
        nc.gpsimd.dma_start(out=bp_bc, in_=_bp_bcast_ap(bass, bp_d))
        wvT = sing.tile([P, NCC, 2, NHD, Dh], BF16, name="wvT")
        nc.gpsimd.dma_start(out=wvT, in_=wv_d[:])
        wpT = sing.tile([P, NHD, C], BF16, name="wpT")
        nc.gpsimd.dma_start(out=wpT, in_=wp_d[:])

        # ---- double-buffered per-chunk tensors ----
        def mk2(shape, name):
            return [sing.tile(shape, BF16, name=f"{name}{i}") for i in range(2)]

        xT2 = mk2([P, NCC, chunk_tok], "xT")
        x8T2 = [sing.tile([P, NCC, chunk_tok], F8, name=f"x8T{i}")
                for i in range(2)]
        qT2 = mk2([P, NHD, chunk_tok], "qT")
        ktbd2 = mk2([P, NHD, NB, P], "ktbd")
        vbd2 = mk2([P, NHD, NB, 2 * (Dh + 1)], "vbd")
        # structural zeros/ones come pre-baked from the host; buffer 0 is
        # chunk-0-critical, buffer 1 only matters by chunk 1
        nc.scalar.dma_start(out=ktbd2[0], in_=kz_d[:])
        nc.scalar.dma_start(out=vbd2[0], in_=vz_d[:])
        nc.gpsimd.dma_start(out=ktbd2[1], in_=kz_d[:])
        nc.gpsimd.dma_start(out=vbd2[1], in_=vz_d[:])

        for ci in range(n_chunks):
            tok0 = ci * chunk_tok
            xT = xT2[ci % 2]
            x8T = x8T2[ci % 2]
            qT = qT2[ci % 2]
            ktbd = ktbd2[ci % 2]
            vbd = vbd2[ci % 2]

            # ---- P0: load pre-transposed X chunk (fp8 for QK, bf16 for V) ----
            for cc in range(NCC):
                nc.sync.dma_start(
                    out=x8T[:, cc, :],
                    in_=x8_d[cc * P:(cc + 1) * P, tok0:tok0 + chunk_tok])
            for cc in range(NCC):
                nc.sync.dma_start(
                    out=xT[:, cc, :],
                    in_=x_d[cc * P:(cc + 1) * P, tok0:tok0 + chunk_tok])

            # ---- P1a: KT / QT projections (fp8 DoubleRow, N=chunk) ----
            for wT, dst in ((wkT, "k"), (wqT, "q")):
                for m in range(NHD):
                    pss = ptile(ps_p, P, (P, chunk_tok), f"proj_{dst}{m}")
                    for cb in range(NCC // 2):
                        nc.tensor.matmul(
                            pss, wT[:, 2 * cb:2 * cb + 2, m * P:(m + 1) * P],
                            x8T[:, 2 * cb:2 * cb + 2, :],
                            start=(cb == 0), stop=(cb == NCC // 2 - 1),
                            perf_mode=mybir.MatmulPerfMode.DoubleRow)
                    if dst == "q":
                        nc.scalar.copy(out=qT[:, m, :], in_=pss)
                    else:
                        # split the two block-diag copies across engines
                        nc.scalar.copy(
                            out=ktbd[0:T, m, :, 0:T],
                            in_=pss[0:T].rearrange("p (nb t) -> p nb t", nb=NB))
                        nc.vector.tensor_copy(
                            out=ktbd[T:P, m, :, T:P],
                            in_=pss[T:P].rearrange("p (nb t) -> p nb t", nb=NB))

            # ---- P1b: V directly in block layout (col-tiled matmul pair) ----
            vbd_v = vbd.rearrange("p a nb (two c) -> p a nb two c", two=2)
            for b in range(NB):
                vpsE = ptile(ps_p, P, (P, NHD, Dh), f"vpsE{b}")
                vpsO = ptile(ps_p, P, (P, NHD, Dh), f"vpsO{b}")
                lhs = xT[:, :, b * T:(b + 1) * T]
                for cc in range(NCC):
                    nc.tensor.matmul(
                        vpsE[0:T], lhs[:, cc, :], wvT[:, cc, 0],
                        start=(cc == 0), stop=(cc == NCC - 1))
                    nc.tensor.matmul(
                        vpsO[T:P], lhs[:, cc, :], wvT[:, cc, 1],
                        start=(cc == 0), stop=(cc == NCC - 1),
                        tile_position=(0, T))
                nc.vector.tensor_copy(
                    out=vbd_v[0:T, :, b, 0, 0:Dh], in_=vpsE[0:T])
                nc.vector.tensor_copy(
                    out=vbd_v[T:P, :, b, 1, 0:Dh], in_=vpsO[T:P])

            # ---- P2+P3: attention, Y PE-transpose, output projection ----
            for it in range(TT):
                yb = yb_p.tile([P, HD], BF16, tag="yb")
                pex = pexp_p.tile([P, 2, NHD, T], BF16, tag="pex", name="pex")
                y_ps = [ptile(ps_y, P, (P, 3, 2 * (Dh + 1)), f"y_ps{h2}")
                        for h2 in range(2)]
                for half in range(2):          # two batches per 128-token tile
                    b = it * 2 + half
                    prow = half * T
                    s_ps = ptile(ps_s, P, (P, NHD, T), f"s_ps{half}")
                    for p_ in range(NHD):
                        nc.tensor.matmul(
                            s_ps[:, p_, :],
                            ktbd[:, p_, b, :],
                            qT[:, p_, b * T:(b + 1) * T],
                            start=True, stop=True)
                    # q,k carry a 16x host-side weight scale each: 0.125/256
                    nc.scalar.activation(
                        out=pex[:, half], in_=s_ps, func=AF.Exp,
                        scale=0.125 / 256.0)
                    nc.vector.tensor_tensor(
                        pex[:, half], pex[:, half],
                        mask_sb[:, None, :].to_broadcast([P, NHD, T]),
                        ALU.mult)
                    for p_ in range(NHD):
                        nc.tensor.matmul(
                            y_ps[p_ // 3][prow:prow + T, p_ % 3, :],
                            pex[:, half, p_, :],
                            vbd[:, p_, b, :],
                            start=True, stop=True)
                for h2 in range(2):
                    y_v = y_ps[h2].rearrange("p a (two c) -> p a two c", c=Dh + 1)
                    rec = small.tile([P, 3, 2, 1], F32, tag="rec", name="rec")
                    nc.vector.reciprocal(out=rec, in_=y_v[:, :, :, Dh:Dh + 1])
                    nc.vector.tensor_tensor(
                        yb[:, h2 * 384:(h2 + 1) * 384]
                            .rearrange("p (a two b) -> p a two b", a=3, two=2),
                        y_v[:, :, :, 0:Dh],
                        rec.to_broadcast([P, 3, 2, Dh]),
                        ALU.mult)
                # Y transpose on PE into one bf16 psum bank
                yt_ps = ptile(ps_t, P, (P, NHD, P), "yt_ps", BF16)
                for j in range(NHD):
                    nc.tensor.transpose(
                        yt_ps[:, j, :], yb[:, j * P:(j + 1) * P], id_sb)
                ytile = yt_p.tile([P, NHD, P], BF16, tag="ytile")
                nc.scalar.copy(out=ytile, in_=yt_ps)
                # output projection
                oA = ptile(ps_o, P, (P, 512), "o_psA")
                oB = ptile(ps_o, P, (P, 256), "o_psB")
                for j in range(NHD):
                    lhs = ytile[:, j, :]
                    nc.tensor.matmul(oA, lhs, wpT[:, j, 0:512],
                                     start=(j == 0), stop=(j == NHD - 1))
                    nc.tensor.matmul(oB, lhs, wpT[:, j, 512:768],
                                     start=(j == 0), stop=(j == NHD - 1))
                osb = ostage.tile([P, C], F32, tag="osb")
                nc.vector.tensor_tensor(osb[:, 0:512], oA, bp_bc[:, 0:512], ALU.add)
                nc.vector.tensor_tensor(osb[:, 512:768], oB, bp_bc[:, 512:768], ALU.add)
                row0 = tok0 + it * P
                nc.sync.dma_start(out=of[row0:row0 + P, :], in_=osb)

    _split_multi_waits(nc, mybir)
    return nc


def _get_program(B_loc, chunk_tok):
    key = (B_loc, chunk_tok)
    if key not in _cache:
        _cache[key] = build_nc(B_loc, chunk_tok)
    return _cache[key]


def make_const_inputs():
    import ml_dtypes
    ident = np.eye(P, dtype=ml_dtypes.bfloat16)
    # mask[s, t] = 1 if s <= t (causal, scoresT layout)
    m = np.tril(np.ones((T, T), dtype=np.float32)).T.astype(ml_dtypes.bfloat16)
    mask = np.vstack([m, m])   # replicated for both head partition-halves
    return ident, mask


def _prep_wT(W):
    """W [768(out), 768(in)] -> wT[p, cc, m] = W[m, cc*128+p], bf16."""
    import ml_dtypes
    w = np.ascontiguousarray(W, dtype=np.float32).reshape(HD, NCC, P)
    return np.ascontiguousarray(
        w.transpose(2, 1, 0)).astype(ml_dtypes.bfloat16)


def prepare(x, Wq, Wk, Wv, Wp, bp, chunk_tok=512):
    import ml_dtypes
    F8NP = ml_dtypes.float8_e4m3
    B_loc = B // N_CORES
    ident, mask = make_const_inputs()
    # Q/K weights in fp8 e4m3, scaled x16 to land in e4m3's normal range;
    # the 16*16 factor is divided back out in the exp() scale.
    w = np.ascontiguousarray(Wq.reshape(HD, C), dtype=np.float32).reshape(HD, NCC, P)
    wqT = np.ascontiguousarray((w * 16.0).transpose(2, 1, 0)).astype(F8NP)
    w = np.ascontiguousarray(Wk.reshape(HD, C), dtype=np.float32).reshape(HD, NCC, P)
    wkT = np.ascontiguousarray((w * 16.0).transpose(2, 1, 0)).astype(F8NP)
    # wvT pre-split by head parity: [p, cc, par, hp, d]
    wvT = _prep_wT(Wv.reshape(HD, C)).reshape(P, NCC, NHD, 2, Dh)
    wvT = np.ascontiguousarray(wvT.transpose(0, 1, 3, 2, 4))
    wpT = _prep_wT(Wp)   # Wp [C_out, HD_in]: contraction on hd
    # per-core x: [B_loc*T, C] -> transposed [C, B_loc*T], bf16 + fp8 copies
    xr = np.asarray(x, dtype=np.float32).reshape(N_CORES, B_loc * T, C)
    xTf = np.ascontiguousarray(xr.transpose(0, 2, 1))
    xTh = xTf.astype(ml_dtypes.bfloat16)
    xT8 = xTf.astype(F8NP)
    # host-baked structural init for the block-diagonal K / V-aug tiles
    NB = chunk_tok // T
    ktbd_init = np.zeros((P, NHD, NB, P), dtype=ml_dtypes.bfloat16)
    vbd_init = np.zeros((P, NHD, NB, 2 * (Dh + 1)), dtype=ml_dtypes.bfloat16)
    vbd_init[0:T, :, :, Dh] = 1.0
    vbd_init[T:P, :, :, 2 * Dh + 1] = 1.0
    nc = _get_program(B_loc, chunk_tok)
    in_maps = []
    for c in range(N_CORES):
        in_maps.append({
            "xT": xTh[c], "xT8": xT8[c],
            "wqT": wqT, "wkT": wkT, "wvT": wvT, "wpT": wpT,
            "bp": np.ascontiguousarray(bp, dtype=np.float32),
            "ident": ident,
            "mask": mask,
            "ktbd_init": ktbd_init, "vbd_init": vbd_init,
        })
    return nc, in_maps


def kernel(x, Wq, Wk, Wv, Wp, bp):
    from concourse import bass_utils

    nc, in_maps = prepare(x, Wq, Wk, Wv, Wp, bp)
    res = bass_utils.run_bass_kernel_spmd(nc, in_maps, list(range(N_CORES)))
    return np.concatenate([res.results[c]["out"] for c in range(N_CORES)], axis=0)
